# revision 43
# baseline (speedup 1.0000x reference)
"""Trainium2 Bass kernel for AdvancedNeuralMemory (B=4, S=8192, D=1024, M=512).

Math
----
s_t = g*s_{t-1} + u_t with scalar g = sigmoid(forget_factor) ~ 0.525.
g^129 < fp32 eps, so mem for a 128-row tile is exactly
    mem_i = Tprev.T @ u_{i-1} + Tcur.T @ u_i
with host-precomputed decay-Toeplitz matrices (adaptive_lr folded in).
Sequential scan -> pure matmuls; 8 cores = (batch 0..3) x (seq half 0..1),
each works a [4096,1024] slab + one 128-row halo tile. No cross-core comm.

V3 design (from trace analysis of the V2 baseline @ 266us):
V2 was ACT/DVE-evacuation-bound with the PE near its cycle floor. Changes:
 * u = v - pred computed by PE accumulation: W2 is negated host-side and
   pred's matmuls accumulate onto the zv PSUM bank, so u needs a single
   PSUM->SBUF fp8 copy (the v stash + DVE subtract of V2 are gone).
 * all 33 u tiles live in one contiguous SBUF buffer, so the two mem
   matmuls merge into ONE DoubleRow matmul (lhsT = [Tprev;Tcur] stacked,
   rhs = u[t-1..t+1]): -512 PE cycles/tile and no tensor pool churn.
 * LN: zq/zk stashed bf16 (one [TS,2,M] tile), ONE segmented bn_stats op
   covers both, aggregates collect into a per-group buffer, and a single
   batched Newton-rsqrt serves the whole group (V2 ran 7 tiny DVE ops per
   tile; now per group).
 * x residual input and y output are bf16: halves their DMA traffic.
"""

import sys
import os

for _p in ("/opt/trn_rl_repo",):
    if _p not in sys.path and os.path.isdir(_p):
        sys.path.insert(0, _p)

from contextlib import ExitStack

import numpy as np
import ml_dtypes

import concourse.bass as bass
import concourse.mybir as mybir
import concourse.tile as tile
from concourse.bass_utils import run_bass_kernel_spmd

B, S, D, M = 4, 8192, 1024, 512
HALF = S // 2          # rows per core
TS = 128               # s-tile rows
NT = HALF // TS        # compute tiles per core (32)
SLAB = HALF + TS       # slab rows incl. halo tile
LN_EPS = 1e-5
N_CORES = 8
GW = 9                 # max tiles per macro-group (sizes 8,8,8,9)
# merge the single-tile tail into the last group: one less pipeline drain
GROUPS = [(0, 8), (8, 8), (16, 8), (24, 9)]
NTILES = NT + 1        # 33 incl. halo

f32 = mybir.dt.float32
bf = mybir.dt.bfloat16
fp8 = mybir.dt.float8e4
u32 = mybir.dt.uint32
AF = mybir.ActivationFunctionType
ALU = mybir.AluOpType
DR = mybir.MatmulPerfMode.DoubleRow
np_bf16 = ml_dtypes.bfloat16
np_fp8 = ml_dtypes.float8_e4m3
SCL = 64.0            # mem scaling so fp8 operands sit in normal range

# packed bf16 weights: wd(8*512) wq/wk/wv/w1/w2(4*512) wu(4*1024) tt(2*128)
WPACK_COLS = 8 * M + 5 * 4 * M + 4 * D + 2 * TS

TRACE = False
TRACE_KWARGS = {}
LAST_RESULTS = None

_PROG_CACHE = {}

MAGIC = 0x5F3759DF


def _fix_matmult_waits(nc):
    """Walrus allows only one sync-wait on a (fused-ldweights) Matmult.
    Move surplus waits onto an inserted NoOp on the same engine."""
    n = 0
    for f in nc.m.functions:
        for bb in f.blocks:
            insts = bb.instructions
            i = 0
            while i < len(insts):
                inst = insts[i]
                si = inst.sync_info
                tname = type(inst).__name__
                exempt = tname in ("InstNoOp",
                                   "InstEventSemaphore",
                                   "InstUnconditionalBranch", "InstCall",
                                   "InstISA", "InstRegisterMove")
                if (not exempt and si is not None and si.on_wait
                        and len(si.on_wait) > 1):
                    for w in list(si.on_wait[:-1]):
                        nop = mybir.InstNoOp(
                            name=f"wfix-{n}", ins=[], outs=[],
                            engine=inst.engine,
                            sync_info=mybir.SyncInfo(on_wait=[w],
                                                     on_update=[]))
                        insts.insert(i, nop)
                        n += 1
                        i += 1
                    si.on_wait = [si.on_wait[-1]]
                i += 1
    return n


def _build_program(flags):
    (has_bd, has_bq, has_bk, has_bv, has_gq, has_bqln, has_gk, has_bkln,
     has_bu) = flags
    nc = bass.Bass()

    # host-pretransposed x, fp8, block-major: [128, 8*SLAB], col = k*SLAB+s
    x_t8 = nc.declare_dram_parameter("x_t8", [TS, 8 * SLAB], fp8,
                                     isOutput=False)
    x_bf = nc.declare_dram_parameter("x_bf", [HALF, D], bf, isOutput=False)
    wpack = nc.declare_dram_parameter("wpack", [TS, WPACK_COLS], fp8,
                                      isOutput=False)
    ident = nc.declare_dram_parameter("ident", [TS, TS], bf, isOutput=False)
    hmask = nc.declare_dram_parameter("hmask", [TS, 1], f32, isOutput=False)
    opt = {}
    for name, used, shape in (
        ("bd_c", has_bd, [TS, 4]), ("bq_b", has_bq, [TS, M]),
        ("bk_b", has_bk, [TS, M]), ("bv_b", has_bv, [TS, M]),
        ("gq_b", has_gq, [TS, M]), ("bqln_b", has_bqln, [TS, M]),
        ("gkT", has_gk, [TS, 4]), ("bklnT", has_bkln, [TS, 4]),
        ("bu_b", has_bu, [TS, D]),
    ):
        if used:
            opt[name] = nc.declare_dram_parameter(name, shape, f32,
                                                  isOutput=False)
    y = nc.declare_dram_parameter("y", [HALF, D], bf, isOutput=True)

    with tile.TileContext(nc) as tc, ExitStack() as ctx:
        wpool = ctx.enter_context(tc.tile_pool(name="weights", bufs=1))

        wp_sb = wpool.tile([TS, WPACK_COLS], fp8)
        # Wd (first 8*M cols) loads alone so the first hT matmul doesn't
        # wait for the whole weight pack; the rest is issued right after
        # group 0's xT DMA (inside phase_a).
        WD_COLS = 8 * M
        nc.sync.dma_start(wp_sb[:, 0:WD_COLS], wpack[:, 0:WD_COLS])
        _off = [0]

        def _wseg(nk, ncols):
            a = _off[0]
            _off[0] += nk * ncols
            return wp_sb[:, a:_off[0]].rearrange("p (k m) -> p k m", k=nk)

        wd_sb = _wseg(8, M)     # [128, 8, 512]: d-blk k -> Wd[d-blk, :]
        wq_sb = _wseg(4, M)
        wk_sb = _wseg(4, M)
        wv_sb = _wseg(4, M)
        w1_sb = _wseg(4, M)
        w2_sb = _wseg(4, M)     # NEGATED host-side (pred accumulates as -pred)
        wu_sb = _wseg(4, D)
        tt_sb = _wseg(2, TS)    # [128, 2, 128] = [Tprev; Tcur] DR-stationary
        id_sb = wpool.tile([TS, TS], bf)
        nc.sync.dma_start(id_sb[:], ident[:])
        hm_sb = wpool.tile([TS, 1], f32)
        nc.sync.dma_start(hm_sb[:], hmask[:])
        # all 33 u tiles contiguous: slot t = u of tile t
        u_all = wpool.tile([TS, NTILES * M], fp8)
        opt_sb = {}
        for name, h in opt.items():
            t = wpool.tile([TS, h.shape[1]], f32, tag=name, name=name)
            nc.sync.dma_start(t[:], h[:])
            opt_sb[name] = t

        # SBUF activation pools
        p_xf = ctx.enter_context(tc.tile_pool(name="xf", bufs=12))
        p_xT = ctx.enter_context(tc.tile_pool(name="xT", bufs=2))
        p_hT = ctx.enter_context(tc.tile_pool(name="hT", bufs=2))
        p_kT = ctx.enter_context(tc.tile_pool(name="kT", bufs=2))
        p_a1T = ctx.enter_context(tc.tile_pool(name="a1T", bufs=2))
        p_q = ctx.enter_context(tc.tile_pool(name="q", bufs=18))
        p_zs = ctx.enter_context(tc.tile_pool(name="zs", bufs=24))
        p_dg = ctx.enter_context(tc.tile_pool(name="dg", bufs=13))
        p_rt = ctx.enter_context(tc.tile_pool(name="rt", bufs=3))
        p_y = ctx.enter_context(tc.tile_pool(name="y", bufs=4))
        p_sm = ctx.enter_context(tc.tile_pool(name="sm", bufs=8))
        p_ln = ctx.enter_context(tc.tile_pool(name="ln", bufs=6))
        # PSUM: pt(2 bf16 banks) + mm(4) + out(2) = 8 banks
        p_pt = ctx.enter_context(tc.tile_pool(name="pt", bufs=2,
                                              space="PSUM"))
        p_mm = ctx.enter_context(tc.tile_pool(name="mm", bufs=4,
                                              space="PSUM"))
        p_out = ctx.enter_context(
            tc.tile_pool(name="out", bufs=2, space="PSUM"))

        def rsqrt_dve(rs_out, xv, nlan, tag):
            """rs_out[128, nlan] f32 = rsqrt(xv) on DVE only."""
            # seed y0 = bits(MAGIC - bits(x)/2); integer ALU on DVE is not
            # available, so do the bit arithmetic in float value domain
            # (|error| < 128 ulp of bit-space -- Newton absorbs it).
            yv = p_sm.tile([TS, nlan], f32, tag=f"{tag}_y")
            t1 = p_sm.tile([TS, nlan], f32, tag=f"{tag}_t")
            t2 = p_sm.tile([TS, nlan], f32, tag=f"{tag}_t2")
            nc.vector.tensor_copy(t1[:], xv[:].bitcast(u32))
            nc.vector.tensor_scalar(t2[:], t1[:], -0.5, float(MAGIC),
                                    ALU.mult, ALU.add)
            nc.vector.tensor_copy(yv[:].bitcast(u32), t2[:])
            # y <- y*(1.5 - 0.5*x*y^2), final result into rs_out
            nc.vector.tensor_mul(t1[:], yv[:], yv[:])
            nc.vector.scalar_tensor_tensor(t1[:], t1[:], -0.5, xv[:],
                                           ALU.mult, ALU.mult)
            nc.vector.tensor_scalar_add(t1[:], t1[:], 1.5)
            nc.vector.tensor_mul(rs_out[:], yv[:], t1[:])

        # state carried between phases
        WMAX = GW * TS

        def phase_a(g0, gn):
            W = gn * TS
            # xT straight from DRAM (host already transposed + fp8-cast)
            xT = p_xT.tile([TS, 8 * WMAX], fp8, tag="xT")
            s0 = g0 * TS
            src = x_t8[:, :].rearrange("p (k s) -> p k s", k=8)[
                :, :, s0:s0 + W]
            dst = xT.rearrange("p (k w) -> p k w", k=8)[:, :, 0:W]
            if g0 == 0:
                # chunked so hT can start after the first 512 tokens land;
                # rest of the weight pack queues behind the first chunk
                for ci, sh in enumerate(range(0, W, 512)):
                    Wc = min(512, W - sh)
                    nc.sync.dma_start(dst[:, :, sh:sh + Wc],
                                      src[:, :, sh:sh + Wc])
                    if ci == 0:
                        nc.sync.dma_start(wp_sb[:, WD_COLS:],
                                          wpack[:, WD_COLS:])
            else:
                nc.sync.dma_start(dst, src)

            # hT[m, s] += Wd[d,m].T @ xT[d, s]  (DoubleRow: K=256/mm)
            # s split at 512 (DR moving operand is 2x the out width)
            xTr = xT.rearrange("p (k w) -> p k w", k=8)
            hT = p_hT.tile([TS, 4 * WMAX], fp8, tag="hT")
            # g-outer: repeated LDWEIGHTS of the same stationary are cheap
            # (~216 vs ~361 ns effective), so reuse each wd block across
            # all s-chunks before moving on. Group 0 instead goes
            # chunk-outer so the first xT chunk's arrival starts compute.
            def ht_chunk(mb, accs):
                for g in range(4):
                    for (acc, sh, Wc) in accs:
                        nc.tensor.matmul(
                            acc[:, 0:Wc],
                            wd_sb[:, 2 * g:2 * g + 2, mb * TS:(mb + 1) * TS],
                            xTr[:, 2 * g:2 * g + 2, sh:sh + Wc],
                            start=(g == 0), stop=(g == 3), perf_mode=DR)
                for (acc, sh, Wc) in accs:
                    if has_bd:
                        nc.vector.tensor_scalar_add(
                            acc[:, 0:Wc], acc[:, 0:Wc],
                            opt_sb["bd_c"][:, mb:mb + 1])
                    nc.scalar.copy(
                        hT[:, mb * WMAX + sh:mb * WMAX + sh + Wc],
                        acc[:, 0:Wc])

            def ht_acc(sh):
                Wc = min(512, W - sh)
                return (p_mm.tile([TS, 4 * TS], f32, tag="mm",
                                  name="hT_ps"), sh, Wc)

            for mb in range(4):
                ht_chunk(mb, [ht_acc(sh) for sh in range(0, W, 512)])

            # per-group LN stats buffers: plane 0 = means, plane 1 = vars,
            # column 2j = q of tile j, 2j+1 = k of tile j
            gag = p_ln.tile([TS, 2, 2 * GW], f32, tag="gag")
            stashes = []
            for j in range(gn):
                t_idx = g0 + j
                halo = (t_idx == 0)

                hTr = hT.rearrange("p (k w) -> p k w", k=4)

                zq = None
                zk = p_mm.tile([TS, M], f32, tag="mm", name="zk")
                if not halo:
                    zq = p_mm.tile([TS, M], f32, tag="mm", name="zq")
                # shared stationary (hT block) between zk and zq per g
                for g in range(2):
                    nc.tensor.matmul(
                        zk[:, 0:M],
                        hTr[:, 2 * g:2 * g + 2, j * TS:(j + 1) * TS],
                        wk_sb[:, 2 * g:2 * g + 2, 0:M],
                        start=(g == 0), stop=(g == 1), perf_mode=DR)
                    if zq is not None:
                        nc.tensor.matmul(
                            zq[:, 0:M],
                            hTr[:, 2 * g:2 * g + 2, j * TS:(j + 1) * TS],
                            wq_sb[:, 2 * g:2 * g + 2, 0:M],
                            start=(g == 0), stop=(g == 1), perf_mode=DR)
                if has_bk:
                    nc.vector.tensor_add(zk[:], zk[:], opt_sb["bk_b"][:])
                if zq is not None and has_bq:
                    nc.vector.tensor_add(zq[:], zq[:], opt_sb["bq_b"][:])

                # bf16 stash [q | k]; frees the PSUM banks, feeds stats+apply
                zs = p_zs.tile([TS, 2, M], bf, tag="zs")
                if zq is not None:
                    nc.scalar.copy(zs[:, 0, :], zq[:])
                else:
                    nc.scalar.copy(zs[:, 0, :], zk[:])  # dummy q = k (halo)
                nc.scalar.copy(zs[:, 1, :], zk[:])
                stashes.append(zs)
                st12 = p_sm.tile([TS, 2, 6], f32, tag="bnst")
                nc.vector.bn_stats(st12[:, 0, :], zs[:, 0, :])
                nc.vector.bn_stats(st12[:, 1, :], zs[:, 1, :])
                nc.vector.bn_aggr(gag[:, :, 2 * j:2 * j + 1], st12[:, 0, :])
                nc.vector.bn_aggr(gag[:, :, 2 * j + 1:2 * j + 2],
                                  st12[:, 1, :])

            return dict(g0=g0, gn=gn, W=W, hT=hT, gag=gag, stashes=stashes)

        def phase_a3(st):
            """Batched Newton rsqrt + diag(rs_k) builds (+ optional q
            apply when gq/bq_ln are nontrivial). Emitted AFTER
            phase_b(prev) so B(prev)'s DVE/ACT evacuations are not stuck
            behind this group's LN chain in the engine FIFOs.

            Wq/Wk are mean-folded host-side, so LN's mean term vanishes:
            q = zq * rs_q, k = zk * rs_k (pure per-token scales)."""
            g0, gn, gag = st["g0"], st["gn"], st["gag"]
            nl = 2 * gn
            xv = p_sm.tile([TS, nl], f32, tag="lnxv")
            nc.vector.tensor_scalar_add(xv[:], gag[:, 1, 0:nl],
                                        float(LN_EPS))
            rs = p_ln.tile([TS, nl], f32, tag="rs")
            rsqrt_dve(rs, xv, nl, "ln")
            st["rs"] = rs
            diags, qs = [], []
            for j in range(gn):
                t_idx = g0 + j
                halo = (t_idx == 0)
                # diag(rs_k): the k LN-apply rides the kT transpose matmul
                dg = p_dg.tile([TS, TS], bf, tag="dg")
                nc.vector.tensor_scalar_mul(dg[:], id_sb[:],
                                            rs[:, 2 * j + 1:2 * j + 2])
                diags.append(dg)
                if halo or not (has_gq or has_bqln):
                    qs.append(None)
                    continue
                zs = st["stashes"][j]
                q_sb = p_q.tile([TS, M], bf, tag="q", name="q")
                nc.vector.tensor_scalar_mul(q_sb[:], zs[:, 0, :],
                                            rs[:, 2 * j:2 * j + 1])
                if has_gq:
                    nc.vector.tensor_mul(q_sb[:], q_sb[:], opt_sb["gq_b"][:])
                if has_bqln:
                    nc.vector.tensor_add(q_sb[:], q_sb[:],
                                         opt_sb["bqln_b"][:])
                qs.append(q_sb)
            st["qs"] = qs
            st["diags"] = diags

        def phase_a2(st):
            """kT for the whole group: scale-transpose via a REGULAR
            matmul zk_stash.T @ diag(rs_k) -- LN apply costs nothing."""
            gn = st["gn"]
            kT = p_kT.tile([TS, 4 * WMAX], fp8, tag="kT")
            for j in range(gn):
                ps = p_pt.tile([TS, 4 * TS], f32, tag="pt", name="ps_k")
                zs = st["stashes"][j]
                dg = st["diags"][j]
                for mb in range(4):
                    nc.tensor.matmul(ps[:, mb * TS:(mb + 1) * TS],
                                     zs[:, 1, mb * TS:(mb + 1) * TS],
                                     dg[:], start=True, stop=True)
                dst = kT.rearrange("p (k w) -> p k w", k=4)[
                    :, :, j * TS:(j + 1) * TS]
                src = ps[:].rearrange("p (k w) -> p k w", k=4)
                if has_gk or has_bkln:
                    for mb in range(4):
                        nc.scalar.activation(
                            kT[:, mb * WMAX + j * TS:
                               mb * WMAX + (j + 1) * TS],
                            ps[:, mb * TS:(mb + 1) * TS],
                            AF.Identity,
                            bias=opt_sb["bklnT"][:, mb:mb + 1]
                            if has_bkln else None,
                            scale=opt_sb["gkT"][:, mb:mb + 1]
                            if has_gk else 1.0)
                else:
                    nc.scalar.copy(dst, src)
            st["kT"] = kT

        def phase_b(st):
            g0, gn, W = st["g0"], st["gn"], st["W"]
            xfs = []
            for j in range(gn):
                t_idx = g0 + j
                if t_idx > 0:
                    xf = p_xf.tile([TS, D], bf, tag="xf")
                    nc.sync.dma_start(
                        xf[:], x_bf[(t_idx - 1) * TS:t_idx * TS, :])
                    xfs.append(xf)
                else:
                    xfs.append(None)
            kTr = st["kT"].rearrange("p (k w) -> p k w", k=4)
            hTr = st["hT"].rearrange("p (k w) -> p k w", k=4)
            # a1T[m1, s] = gelu(W1[m,m1].T @ kT[m, s])
            a1T = p_a1T.tile([TS, 4 * WMAX], fp8, tag="a1T")
            for m1b in range(4):
                accs = []
                for sh in range(0, W, 512):
                    Wc = min(512, W - sh)
                    accs.append((p_mm.tile([TS, 4 * TS], f32, tag="mm",
                                           name="a1_ps"), sh, Wc))
                for g in range(2):
                    for (acc, sh, Wc) in accs:
                        nc.tensor.matmul(
                            acc[:, 0:Wc],
                            w1_sb[:, 2 * g:2 * g + 2,
                                  m1b * TS:(m1b + 1) * TS],
                            kTr[:, 2 * g:2 * g + 2, sh:sh + Wc],
                            start=(g == 0), stop=(g == 1), perf_mode=DR)
                for (acc, sh, Wc) in accs:
                    nc.scalar.activation(
                        a1T[:, m1b * WMAX + sh:m1b * WMAX + sh + Wc],
                        acc[:, 0:Wc], AF.Gelu_apprx_tanh)

            # breadth-first over the group's tiles so each PE stage's DVE
            # dependencies were produced a stage earlier
            a1Tr = a1T.rearrange("p (k w) -> p k w", k=4)
            rtrs, rTs = [], []
            for j in range(gn):
                t_idx = g0 + j
                # u = v - pred, accumulated on the PE: zv mms then pred
                # mms with host-negated W2 into the SAME PSUM bank.
                ups = p_mm.tile([TS, M], f32, tag="mm", name="ups")
                for g in range(2):
                    nc.tensor.matmul(
                        ups[:, 0:M],
                        hTr[:, 2 * g:2 * g + 2, j * TS:(j + 1) * TS],
                        wv_sb[:, 2 * g:2 * g + 2, 0:M],
                        start=(g == 0), stop=False, perf_mode=DR)
                for g in range(2):
                    nc.tensor.matmul(
                        ups[:, 0:M],
                        a1Tr[:, 2 * g:2 * g + 2, j * TS:(j + 1) * TS],
                        w2_sb[:, 2 * g:2 * g + 2, 0:M],
                        start=False, stop=(g == 1), perf_mode=DR)
                u_dst = u_all[:, t_idx * M:(t_idx + 1) * M]
                if t_idx == 0:
                    if has_bv:
                        nc.vector.tensor_add(u_dst, ups[:],
                                             opt_sb["bv_b"][:])
                        nc.vector.tensor_scalar_mul(u_dst, u_dst,
                                                    hm_sb[:, 0:1])
                    else:
                        nc.vector.tensor_scalar_mul(u_dst, ups[:],
                                                    hm_sb[:, 0:1])
                elif has_bv:
                    nc.vector.tensor_add(u_dst, ups[:], opt_sb["bv_b"][:])
                else:
                    nc.vector.tensor_copy(u_dst, ups[:])
            for j in range(gn):
                t_idx = g0 + j
                if t_idx == 0:
                    rtrs.append(None)
                    continue
                mem = p_mm.tile([TS, M], f32, tag="mm", name="mem")
                # single DoubleRow matmul: [Tprev;Tcur].T @ [u_{t-1}|u_t]
                nc.tensor.matmul(
                    mem[:, 0:M], tt_sb[:, 0:2, :],
                    u_all[:, (t_idx - 1) * M:(t_idx + 1) * M].rearrange(
                        "p (k m) -> p k m", k=2),
                    start=True, stop=True, perf_mode=DR)
                rtr = p_rt.tile([TS, M], bf, tag="rtr")
                if st["qs"][j] is not None:
                    nc.vector.tensor_mul(rtr[:], st["qs"][j][:], mem[:])
                else:
                    # q LN-apply fused in: rtr = (zq * rs_q) * mem
                    nc.vector.scalar_tensor_tensor(
                        rtr[:], st["stashes"][j][:, 0, :],
                        st["rs"][:, 2 * j:2 * j + 1], mem[:],
                        ALU.mult, ALU.mult)
                rtrs.append(rtr)  # 64x-scaled (SCL inside the T matrices)
            for j in range(gn):
                if rtrs[j] is None:
                    rTs.append(None)
                    continue
                ps = p_pt.tile([TS, 4 * TS], bf, tag="pt", name="ps_r")
                for mb in range(4):
                    nc.tensor.transpose(ps[:, mb * TS:(mb + 1) * TS],
                                        rtrs[j][:, mb * TS:(mb + 1) * TS],
                                        id_sb[:])
                rT = p_rt.tile([TS, 4 * TS], fp8, tag="rT")
                nc.scalar.copy(rT[:], ps[:])
                rTs.append(rT)
            for j in range(gn):
                if rTs[j] is None:
                    continue
                t_idx = g0 + j
                rTr = rTs[j].rearrange("p (k w) -> p k w", k=4)
                # g-outer over both 512-col halves: the rT stationary is
                # shared, so its expensive first LDWEIGHTS amortizes
                pss = [p_out.tile([TS, 512], f32, tag="out", name="out_ps")
                       for _ in range(2)]
                for g in range(2):
                    for nb in range(2):
                        nc.tensor.matmul(
                            pss[nb][:],
                            rTr[:, 2 * g:2 * g + 2, 0:TS],
                            wu_sb[:, 2 * g:2 * g + 2,
                                  nb * 512:(nb + 1) * 512],
                            start=(g == 0), stop=(g == 1), perf_mode=DR)
                for nb in range(2):
                    cols = slice(nb * 512, (nb + 1) * 512)
                    y_sb = p_y.tile([TS, 512], bf, tag="y")
                    # y = x + out/SCL  (out carries the 64x mem scaling)
                    nc.vector.scalar_tensor_tensor(
                        y_sb[:], pss[nb][:], 1.0 / SCL, xfs[j][:, cols],
                        ALU.mult, ALU.add)
                    if has_bu:
                        nc.vector.tensor_add(y_sb[:], y_sb[:],
                                             opt_sb["bu_b"][:, cols])
                    nc.sync.dma_start(y[(t_idx - 1) * TS:t_idx * TS, cols],
                                      y_sb[:])

        # software pipeline: K(g-1), A(g), B(g-1), A3(g), ...
        # A3 (newton + diag builds) MUST come after B(g-1): its newton
        # depends on A(g)'s last stats, and anything queued behind it on
        # DVE would stall -- B(g-1)'s u/rtr/y evacuations gate B's PE.
        prev = None
        for (g0, gn) in GROUPS:
            if prev is not None:
                phase_a2(prev)
            cur = phase_a(g0, gn)
            if prev is not None:
                phase_b(prev)
            phase_a3(cur)
            prev = cur
        phase_a2(prev)
        phase_b(prev)

    _fix_matmult_waits(nc)
    return nc


def _prep_inputs(x, Wd, bd, Wq, bq, Wk, bk, Wv, bv, gq, bq_ln, gk, bk_ln,
                 W1, W2, Wu, bu, adaptive_lr, forget_factor):
    """Host-side: flags, decay matrices, per-core slabs, bf16 packing."""
    f = np.float32
    bd, bq, bk, bv, bu = (np.asarray(a, f) for a in (bd, bq, bk, bv, bu))
    gq, bq_ln, gk, bk_ln = (np.asarray(a, f) for a in (gq, bq_ln, gk, bk_ln))
    # mean-fold: LN subtracts the mean, so project Wq/Wk (and bq/bk) onto
    # zero-column-mean space host-side -- the on-device mean term vanishes
    Wq = np.asarray(Wq, f) - np.mean(np.asarray(Wq, f), axis=1, keepdims=True)
    Wk = np.asarray(Wk, f) - np.mean(np.asarray(Wk, f), axis=1, keepdims=True)
    bq = bq - bq.mean()
    bk = bk - bk.mean()
    flags = (bool(bd.any()), bool(bq.any()), bool(bk.any()), bool(bv.any()),
             bool((gq != 1).any()), bool(bq_ln.any()),
             bool((gk != 1).any()), bool(bk_ln.any()), bool(bu.any()))

    g = 1.0 / (1.0 + np.exp(-np.float64(forget_factor)))
    lr = np.float64(adaptive_lr)
    t_idx = np.arange(TS)
    lag_cur = t_idx[:, None] - t_idx[None, :]
    Tcur = np.where(lag_cur >= 0, g ** np.maximum(lag_cur, 0), 0.0) * lr * SCL
    lag_prev = t_idx[:, None] + TS - t_idx[None, :]
    Tprev = (g ** lag_prev) * lr * SCL
    TT = np.concatenate([Tprev, Tcur], axis=1).T.astype(f)  # [256, 128]

    def seg(w):
        w = np.asarray(w, f)          # [K, N] -> [128, nk*N]
        nk = w.shape[0] // TS
        return w.reshape(nk, TS, w.shape[1]).transpose(1, 0, 2).reshape(TS, -1)

    wpack = np.ascontiguousarray(np.concatenate(
        [seg(w) for w in (Wd, Wq, Wk, Wv, W1, -np.asarray(W2, f), Wu, TT)],
        axis=1)).astype(np_fp8)
    common = {
        "wpack": wpack,
        "ident": np.eye(TS, dtype=f).astype(np_bf16),
    }
    names = ("bd_c", "bq_b", "bk_b", "bv_b", "gq_b", "bqln_b", "gkT",
             "bklnT", "bu_b")
    vecs = (bd, bq, bk, bv, gq, bq_ln, gk, bk_ln, bu)
    for name, used, vec in zip(names, flags, vecs):
        if not used:
            continue
        if name in ("bd_c", "gkT", "bklnT"):
            common[name] = np.ascontiguousarray(
                vec.reshape(4, TS).T, f)      # [128, 4]: col mb = block
        else:
            common[name] = np.ascontiguousarray(
                np.broadcast_to(vec, (TS, vec.shape[0])), f)

    x = np.asarray(x, f)
    in_maps = []
    for c in range(N_CORES):
        b, sh = c // 2, c % 2
        if sh == 0:
            haloblk = np.zeros((TS, D), f)
            hm = np.zeros((TS, 1), f)
        else:
            haloblk = x[b, HALF - TS:HALF]
            hm = np.ones((TS, 1), f)
        slab = np.concatenate([haloblk, x[b, sh * HALF:(sh + 1) * HALF]],
                              axis=0)
        m = dict(common)
        # [SLAB, D] -> transpose -> [8, 128, SLAB] -> [128, 8*SLAB] fp8
        xt = np.ascontiguousarray(slab.T).reshape(8, TS, SLAB)
        m["x_t8"] = np.ascontiguousarray(
            xt.transpose(1, 0, 2).reshape(TS, 8 * SLAB)).astype(np_fp8)
        m["x_bf"] = np.ascontiguousarray(
            x[b, sh * HALF:(sh + 1) * HALF]).astype(np_bf16)
        m["hmask"] = hm
        in_maps.append(m)
    return flags, in_maps


def kernel(**inputs):
    global LAST_RESULTS
    flags, in_maps = _prep_inputs(**inputs)
    if flags not in _PROG_CACHE:
        _PROG_CACHE[flags] = _build_program(flags)
    nc = _PROG_CACHE[flags]

    res = run_bass_kernel_spmd(nc, in_maps, list(range(N_CORES)),
                               trace=TRACE, trace_kwargs=TRACE_KWARGS)
    LAST_RESULTS = res

    out = np.empty((B, S, D), np.float32)
    for c in range(N_CORES):
        b, sh = c // 2, c % 2
        out[b, sh * HALF:(sh + 1) * HALF] = res.results[c]["y"].astype(
            np.float32)
    return out


# revision 46
# speedup vs baseline: 1.0471x; 1.0471x over previous
"""Trainium2 Bass kernel for AdvancedNeuralMemory (B=4, S=8192, D=1024, M=512).

Math
----
s_t = g*s_{t-1} + u_t with scalar g = sigmoid(forget_factor) ~ 0.525.
g^129 < fp32 eps, so mem for a 128-row tile is exactly
    mem_i = Tprev.T @ u_{i-1} + Tcur.T @ u_i
with host-precomputed decay-Toeplitz matrices (adaptive_lr folded in).
Sequential scan -> pure matmuls; 8 cores = (batch 0..3) x (seq half 0..1),
each works a [4096,1024] slab + one 128-row halo tile. No cross-core comm.

V3 design (from trace analysis of the V2 baseline @ 266us):
V2 was ACT/DVE-evacuation-bound with the PE near its cycle floor. Changes:
 * u = v - pred computed by PE accumulation: W2 is negated host-side and
   pred's matmuls accumulate onto the zv PSUM bank, so u needs a single
   PSUM->SBUF fp8 copy (the v stash + DVE subtract of V2 are gone).
 * all 33 u tiles live in one contiguous SBUF buffer, so the two mem
   matmuls merge into ONE DoubleRow matmul (lhsT = [Tprev;Tcur] stacked,
   rhs = u[t-1..t+1]): -512 PE cycles/tile and no tensor pool churn.
 * LN: zq/zk stashed bf16 (one [TS,2,M] tile), ONE segmented bn_stats op
   covers both, aggregates collect into a per-group buffer, and a single
   batched Newton-rsqrt serves the whole group (V2 ran 7 tiny DVE ops per
   tile; now per group).
 * x residual input and y output are bf16: halves their DMA traffic.
"""

import sys
import os

for _p in ("/opt/trn_rl_repo",):
    if _p not in sys.path and os.path.isdir(_p):
        sys.path.insert(0, _p)

from contextlib import ExitStack

import numpy as np
import ml_dtypes

import concourse.bass as bass
import concourse.mybir as mybir
import concourse.tile as tile
from concourse.bass_utils import run_bass_kernel_spmd

B, S, D, M = 4, 8192, 1024, 512
HALF = S // 2          # rows per core
TS = 128               # s-tile rows
NT = HALF // TS        # compute tiles per core (32)
SLAB = HALF + TS       # slab rows incl. halo tile
LN_EPS = 1e-5
N_CORES = 8
GW = 9                 # max tiles per macro-group (sizes 8,8,8,9)
# merge the single-tile tail into the last group: one less pipeline drain
GROUPS = [(0, 8), (8, 8), (16, 8), (24, 9)]
NTILES = NT + 1        # 33 incl. halo

f32 = mybir.dt.float32
bf = mybir.dt.bfloat16
fp8 = mybir.dt.float8e4
u32 = mybir.dt.uint32
AF = mybir.ActivationFunctionType
ALU = mybir.AluOpType
DR = mybir.MatmulPerfMode.DoubleRow
np_bf16 = ml_dtypes.bfloat16
np_fp8 = ml_dtypes.float8_e4m3
SCL = 64.0            # mem scaling so fp8 operands sit in normal range

# packed bf16 weights: wd(8*512) wq/wk/wv/w1/w2(4*512) wu(4*1024) tt(2*128)
WPACK_COLS = 8 * M + 5 * 4 * M + 4 * D + 2 * TS

TRACE = False
TRACE_KWARGS = {}
LAST_RESULTS = None

_PROG_CACHE = {}

MAGIC = 0x5F3759DF


def _fix_matmult_waits(nc):
    """Walrus allows only one sync-wait on a (fused-ldweights) Matmult.
    Move surplus waits onto an inserted NoOp on the same engine."""
    n = 0
    for f in nc.m.functions:
        for bb in f.blocks:
            insts = bb.instructions
            i = 0
            while i < len(insts):
                inst = insts[i]
                si = inst.sync_info
                tname = type(inst).__name__
                exempt = tname in ("InstNoOp",
                                   "InstEventSemaphore",
                                   "InstUnconditionalBranch", "InstCall",
                                   "InstISA", "InstRegisterMove")
                if (not exempt and si is not None and si.on_wait
                        and len(si.on_wait) > 1):
                    for w in list(si.on_wait[:-1]):
                        nop = mybir.InstNoOp(
                            name=f"wfix-{n}", ins=[], outs=[],
                            engine=inst.engine,
                            sync_info=mybir.SyncInfo(on_wait=[w],
                                                     on_update=[]))
                        insts.insert(i, nop)
                        n += 1
                        i += 1
                    si.on_wait = [si.on_wait[-1]]
                i += 1
    return n


def _build_program(flags):
    (has_bd, has_bq, has_bk, has_bv, has_gq, has_bqln, has_gk, has_bkln,
     has_bu) = flags
    nc = bass.Bass()

    # host-pretransposed x, fp8, block-major: [128, 8*SLAB], col = k*SLAB+s
    x_t8 = nc.declare_dram_parameter("x_t8", [TS, 8 * SLAB], fp8,
                                     isOutput=False)
    x_bf = nc.declare_dram_parameter("x_bf", [HALF, D], bf, isOutput=False)
    wpack = nc.declare_dram_parameter("wpack", [TS, WPACK_COLS], fp8,
                                      isOutput=False)
    ident = nc.declare_dram_parameter("ident", [TS, TS], bf, isOutput=False)
    hmask = nc.declare_dram_parameter("hmask", [TS, 1], f32, isOutput=False)
    opt = {}
    for name, used, shape in (
        ("bd_c", has_bd, [TS, 4]), ("bq_b", has_bq, [TS, M]),
        ("bk_b", has_bk, [TS, M]), ("bv_b", has_bv, [TS, M]),
        ("gq_b", has_gq, [TS, M]), ("bqln_b", has_bqln, [TS, M]),
        ("gkT", has_gk, [TS, 4]), ("bklnT", has_bkln, [TS, 4]),
        ("bu_b", has_bu, [TS, D]),
    ):
        if used:
            opt[name] = nc.declare_dram_parameter(name, shape, f32,
                                                  isOutput=False)
    y = nc.declare_dram_parameter("y", [HALF, D], bf, isOutput=True)

    with tile.TileContext(nc) as tc, ExitStack() as ctx:
        wpool = ctx.enter_context(tc.tile_pool(name="weights", bufs=1))

        wp_sb = wpool.tile([TS, WPACK_COLS], fp8)
        # Wd (first 8*M cols) loads alone so the first hT matmul doesn't
        # wait for the whole weight pack; the rest is issued right after
        # group 0's xT DMA (inside phase_a).
        WD_COLS = 8 * M
        nc.sync.dma_start(wp_sb[:, 0:WD_COLS], wpack[:, 0:WD_COLS])
        _off = [0]

        def _wseg(nk, ncols):
            a = _off[0]
            _off[0] += nk * ncols
            return wp_sb[:, a:_off[0]].rearrange("p (k m) -> p k m", k=nk)

        wd_sb = _wseg(8, M)     # [128, 8, 512]: d-blk k -> Wd[d-blk, :]
        wq_sb = _wseg(4, M)
        wk_sb = _wseg(4, M)
        wv_sb = _wseg(4, M)
        w1_sb = _wseg(4, M)
        w2_sb = _wseg(4, M)     # NEGATED host-side (pred accumulates as -pred)
        wu_sb = _wseg(4, D)
        tt_sb = _wseg(2, TS)    # [128, 2, 128] = [Tprev; Tcur] DR-stationary
        id_sb = wpool.tile([TS, TS], bf)
        nc.sync.dma_start(id_sb[:], ident[:])
        hm_sb = wpool.tile([TS, 1], f32)
        nc.sync.dma_start(hm_sb[:], hmask[:])
        # all 33 u tiles contiguous: slot t = u of tile t
        u_all = wpool.tile([TS, NTILES * M], fp8)
        opt_sb = {}
        for name, h in opt.items():
            t = wpool.tile([TS, h.shape[1]], f32, tag=name, name=name)
            nc.sync.dma_start(t[:], h[:])
            opt_sb[name] = t

        # SBUF activation pools
        p_xf = ctx.enter_context(tc.tile_pool(name="xf", bufs=12))
        p_xT = ctx.enter_context(tc.tile_pool(name="xT", bufs=2))
        p_hT = ctx.enter_context(tc.tile_pool(name="hT", bufs=2))
        p_kT = ctx.enter_context(tc.tile_pool(name="kT", bufs=2))
        p_a1T = ctx.enter_context(tc.tile_pool(name="a1T", bufs=2))
        p_q = ctx.enter_context(tc.tile_pool(name="q", bufs=18))
        p_zs = ctx.enter_context(tc.tile_pool(name="zs", bufs=24))
        p_dg = ctx.enter_context(tc.tile_pool(name="dg", bufs=13))
        p_rt = ctx.enter_context(tc.tile_pool(name="rt", bufs=3))
        p_y = ctx.enter_context(tc.tile_pool(name="y", bufs=4))
        p_sm = ctx.enter_context(tc.tile_pool(name="sm", bufs=8))
        p_ln = ctx.enter_context(tc.tile_pool(name="ln", bufs=6))
        # PSUM: pt(2 bf16 banks) + mm(4) + out(2) = 8 banks
        p_pt = ctx.enter_context(tc.tile_pool(name="pt", bufs=2,
                                              space="PSUM"))
        p_mm = ctx.enter_context(tc.tile_pool(name="mm", bufs=4,
                                              space="PSUM"))
        p_out = ctx.enter_context(
            tc.tile_pool(name="out", bufs=2, space="PSUM"))

        def rsqrt_dve(rs_out, xv, nlan, tag):
            """rs_out[128, nlan] f32 = rsqrt(xv) on DVE only."""
            # seed y0 = bits(MAGIC - bits(x)/2); integer ALU on DVE is not
            # available, so do the bit arithmetic in float value domain
            # (|error| < 128 ulp of bit-space -- Newton absorbs it).
            yv = p_sm.tile([TS, nlan], f32, tag=f"{tag}_y")
            t1 = p_sm.tile([TS, nlan], f32, tag=f"{tag}_t")
            t2 = p_sm.tile([TS, nlan], f32, tag=f"{tag}_t2")
            nc.vector.tensor_copy(t1[:], xv[:].bitcast(u32))
            nc.vector.tensor_scalar(t2[:], t1[:], -0.5, float(MAGIC),
                                    ALU.mult, ALU.add)
            nc.vector.tensor_copy(yv[:].bitcast(u32), t2[:])
            # y <- y*(1.5 - 0.5*x*y^2), final result into rs_out
            nc.vector.tensor_mul(t1[:], yv[:], yv[:])
            nc.vector.scalar_tensor_tensor(t1[:], t1[:], -0.5, xv[:],
                                           ALU.mult, ALU.mult)
            nc.vector.tensor_scalar_add(t1[:], t1[:], 1.5)
            nc.vector.tensor_mul(rs_out[:], yv[:], t1[:])

        # state carried between phases
        WMAX = GW * TS

        xt_tiles = {}

        def xt_load(gi):
            """Prefetch group gi's xT slab (issued one group early)."""
            if gi >= len(GROUPS) or gi in xt_tiles:
                return
            g0, gn = GROUPS[gi]
            W = gn * TS
            xT = p_xT.tile([TS, 8 * WMAX], fp8, tag="xT", name="xT")
            s0 = g0 * TS
            src = x_t8[:, :].rearrange("p (k s) -> p k s", k=8)[
                :, :, s0:s0 + W]
            dst = xT.rearrange("p (k w) -> p k w", k=8)[:, :, 0:W]
            nc.sync.dma_start(dst, src)
            xt_tiles[gi] = xT

        def phase_a(gi, g0, gn):
            W = gn * TS
            xT = xt_tiles.pop(gi)

            # hT[m, s] += Wd[d,m].T @ xT[d, s]  (DoubleRow: K=256/mm)
            # s split at 512 (DR moving operand is 2x the out width)
            xTr = xT.rearrange("p (k w) -> p k w", k=8)
            hT = p_hT.tile([TS, 4 * WMAX], fp8, tag="hT")
            # g-outer: repeated LDWEIGHTS of the same stationary are cheap
            # (~216 vs ~361 ns effective), so reuse each wd block across
            # all s-chunks before moving on. Group 0 instead goes
            # chunk-outer so the first xT chunk's arrival starts compute.
            def ht_chunk(mb, accs):
                for g in range(4):
                    for (acc, sh, Wc) in accs:
                        nc.tensor.matmul(
                            acc[:, 0:Wc],
                            wd_sb[:, 2 * g:2 * g + 2, mb * TS:(mb + 1) * TS],
                            xTr[:, 2 * g:2 * g + 2, sh:sh + Wc],
                            start=(g == 0), stop=(g == 3), perf_mode=DR)
                for (acc, sh, Wc) in accs:
                    if has_bd:
                        nc.vector.tensor_scalar_add(
                            acc[:, 0:Wc], acc[:, 0:Wc],
                            opt_sb["bd_c"][:, mb:mb + 1])
                    nc.scalar.copy(
                        hT[:, mb * WMAX + sh:mb * WMAX + sh + Wc],
                        acc[:, 0:Wc])

            def ht_acc(sh):
                Wc = min(512, W - sh)
                return (p_mm.tile([TS, 4 * TS], f32, tag="mm",
                                  name="hT_ps"), sh, Wc)

            for mb in range(4):
                ht_chunk(mb, [ht_acc(sh) for sh in range(0, W, 512)])

            # per-group LN stats buffers: plane 0 = means, plane 1 = vars,
            # column 2j = q of tile j, 2j+1 = k of tile j
            gag = p_ln.tile([TS, 2, 2 * GW], f32, tag="gag")
            stashes = []
            for j in range(gn):
                t_idx = g0 + j
                halo = (t_idx == 0)

                hTr = hT.rearrange("p (k w) -> p k w", k=4)

                zq = None
                zk = p_mm.tile([TS, M], f32, tag="mm", name="zk")
                if not halo:
                    zq = p_mm.tile([TS, M], f32, tag="mm", name="zq")
                # shared stationary (hT block) between zk and zq per g
                for g in range(2):
                    nc.tensor.matmul(
                        zk[:, 0:M],
                        hTr[:, 2 * g:2 * g + 2, j * TS:(j + 1) * TS],
                        wk_sb[:, 2 * g:2 * g + 2, 0:M],
                        start=(g == 0), stop=(g == 1), perf_mode=DR)
                    if zq is not None:
                        nc.tensor.matmul(
                            zq[:, 0:M],
                            hTr[:, 2 * g:2 * g + 2, j * TS:(j + 1) * TS],
                            wq_sb[:, 2 * g:2 * g + 2, 0:M],
                            start=(g == 0), stop=(g == 1), perf_mode=DR)
                if has_bk:
                    nc.vector.tensor_add(zk[:], zk[:], opt_sb["bk_b"][:])
                if zq is not None and has_bq:
                    nc.vector.tensor_add(zq[:], zq[:], opt_sb["bq_b"][:])

                # bf16 stash [q | k]; frees the PSUM banks, feeds stats+apply
                zs = p_zs.tile([TS, 2, M], bf, tag="zs")
                if zq is not None:
                    nc.scalar.copy(zs[:, 0, :], zq[:])
                else:
                    nc.scalar.copy(zs[:, 0, :], zk[:])  # dummy q = k (halo)
                nc.scalar.copy(zs[:, 1, :], zk[:])
                stashes.append(zs)
                st12 = p_sm.tile([TS, 2, 6], f32, tag="bnst")
                nc.vector.bn_stats(st12[:, 0, :], zs[:, 0, :])
                nc.vector.bn_stats(st12[:, 1, :], zs[:, 1, :])
                nc.vector.bn_aggr(gag[:, :, 2 * j:2 * j + 1], st12[:, 0, :])
                nc.vector.bn_aggr(gag[:, :, 2 * j + 1:2 * j + 2],
                                  st12[:, 1, :])

            return dict(g0=g0, gn=gn, W=W, hT=hT, gag=gag, stashes=stashes)

        def phase_a3(st):
            """Batched Newton rsqrt + diag(rs_k) builds (+ optional q
            apply when gq/bq_ln are nontrivial). Emitted AFTER
            phase_b(prev) so B(prev)'s DVE/ACT evacuations are not stuck
            behind this group's LN chain in the engine FIFOs.

            Wq/Wk are mean-folded host-side, so LN's mean term vanishes:
            q = zq * rs_q, k = zk * rs_k (pure per-token scales)."""
            g0, gn, gag = st["g0"], st["gn"], st["gag"]
            nl = 2 * gn
            xv = p_sm.tile([TS, nl], f32, tag="lnxv")
            nc.vector.tensor_scalar_add(xv[:], gag[:, 1, 0:nl],
                                        float(LN_EPS))
            rs = p_ln.tile([TS, nl], f32, tag="rs")
            rsqrt_dve(rs, xv, nl, "ln")
            st["rs"] = rs
            diags, qs = [], []
            for j in range(gn):
                t_idx = g0 + j
                halo = (t_idx == 0)
                # diag(rs_k): the k LN-apply rides the kT transpose matmul
                dg = p_dg.tile([TS, TS], bf, tag="dg")
                nc.vector.tensor_scalar_mul(dg[:], id_sb[:],
                                            rs[:, 2 * j + 1:2 * j + 2])
                diags.append(dg)
                if halo or not (has_gq or has_bqln):
                    qs.append(None)
                    continue
                zs = st["stashes"][j]
                q_sb = p_q.tile([TS, M], bf, tag="q", name="q")
                nc.vector.tensor_scalar_mul(q_sb[:], zs[:, 0, :],
                                            rs[:, 2 * j:2 * j + 1])
                if has_gq:
                    nc.vector.tensor_mul(q_sb[:], q_sb[:], opt_sb["gq_b"][:])
                if has_bqln:
                    nc.vector.tensor_add(q_sb[:], q_sb[:],
                                         opt_sb["bqln_b"][:])
                qs.append(q_sb)
            st["qs"] = qs
            st["diags"] = diags

        def phase_a2(st):
            """kT for the whole group: scale-transpose via a REGULAR
            matmul zk_stash.T @ diag(rs_k) -- LN apply costs nothing."""
            gn = st["gn"]
            kT = p_kT.tile([TS, 4 * WMAX], fp8, tag="kT")
            for j in range(gn):
                ps = p_pt.tile([TS, 4 * TS], f32, tag="pt", name="ps_k")
                zs = st["stashes"][j]
                dg = st["diags"][j]
                for mb in range(4):
                    nc.tensor.matmul(ps[:, mb * TS:(mb + 1) * TS],
                                     zs[:, 1, mb * TS:(mb + 1) * TS],
                                     dg[:], start=True, stop=True)
                dst = kT.rearrange("p (k w) -> p k w", k=4)[
                    :, :, j * TS:(j + 1) * TS]
                src = ps[:].rearrange("p (k w) -> p k w", k=4)
                if has_gk or has_bkln:
                    for mb in range(4):
                        nc.scalar.activation(
                            kT[:, mb * WMAX + j * TS:
                               mb * WMAX + (j + 1) * TS],
                            ps[:, mb * TS:(mb + 1) * TS],
                            AF.Identity,
                            bias=opt_sb["bklnT"][:, mb:mb + 1]
                            if has_bkln else None,
                            scale=opt_sb["gkT"][:, mb:mb + 1]
                            if has_gk else 1.0)
                else:
                    nc.scalar.copy(dst, src)
            st["kT"] = kT

        def phase_b(st):
            g0, gn, W = st["g0"], st["gn"], st["W"]
            xfs = []
            for j in range(gn):
                t_idx = g0 + j
                if t_idx > 0:
                    xf = p_xf.tile([TS, D], bf, tag="xf")
                    nc.sync.dma_start(
                        xf[:], x_bf[(t_idx - 1) * TS:t_idx * TS, :])
                    xfs.append(xf)
                else:
                    xfs.append(None)
            kTr = st["kT"].rearrange("p (k w) -> p k w", k=4)
            hTr = st["hT"].rearrange("p (k w) -> p k w", k=4)
            # a1T[m1, s] = gelu(W1[m,m1].T @ kT[m, s])
            a1T = p_a1T.tile([TS, 4 * WMAX], fp8, tag="a1T")
            for m1b in range(4):
                accs = []
                for sh in range(0, W, 512):
                    Wc = min(512, W - sh)
                    accs.append((p_mm.tile([TS, 4 * TS], f32, tag="mm",
                                           name="a1_ps"), sh, Wc))
                for g in range(2):
                    for (acc, sh, Wc) in accs:
                        nc.tensor.matmul(
                            acc[:, 0:Wc],
                            w1_sb[:, 2 * g:2 * g + 2,
                                  m1b * TS:(m1b + 1) * TS],
                            kTr[:, 2 * g:2 * g + 2, sh:sh + Wc],
                            start=(g == 0), stop=(g == 1), perf_mode=DR)
                for (acc, sh, Wc) in accs:
                    nc.scalar.activation(
                        a1T[:, m1b * WMAX + sh:m1b * WMAX + sh + Wc],
                        acc[:, 0:Wc], AF.Gelu_apprx_tanh)

            # breadth-first over the group's tiles so each PE stage's DVE
            # dependencies were produced a stage earlier
            a1Tr = a1T.rearrange("p (k w) -> p k w", k=4)
            rtrs, rTs = [], []
            for j in range(gn):
                t_idx = g0 + j
                # u = v - pred, accumulated on the PE: zv mms then pred
                # mms with host-negated W2 into the SAME PSUM bank.
                ups = p_mm.tile([TS, M], f32, tag="mm", name="ups")
                for g in range(2):
                    nc.tensor.matmul(
                        ups[:, 0:M],
                        hTr[:, 2 * g:2 * g + 2, j * TS:(j + 1) * TS],
                        wv_sb[:, 2 * g:2 * g + 2, 0:M],
                        start=(g == 0), stop=False, perf_mode=DR)
                for g in range(2):
                    nc.tensor.matmul(
                        ups[:, 0:M],
                        a1Tr[:, 2 * g:2 * g + 2, j * TS:(j + 1) * TS],
                        w2_sb[:, 2 * g:2 * g + 2, 0:M],
                        start=False, stop=(g == 1), perf_mode=DR)
                u_dst = u_all[:, t_idx * M:(t_idx + 1) * M]
                if t_idx == 0:
                    if has_bv:
                        nc.vector.tensor_add(u_dst, ups[:],
                                             opt_sb["bv_b"][:])
                        nc.vector.tensor_scalar_mul(u_dst, u_dst,
                                                    hm_sb[:, 0:1])
                    else:
                        nc.vector.tensor_scalar_mul(u_dst, ups[:],
                                                    hm_sb[:, 0:1])
                elif has_bv:
                    nc.vector.tensor_add(u_dst, ups[:], opt_sb["bv_b"][:])
                else:
                    nc.vector.tensor_copy(u_dst, ups[:])
            for j in range(gn):
                t_idx = g0 + j
                if t_idx == 0:
                    rtrs.append(None)
                    continue
                mem = p_mm.tile([TS, M], f32, tag="mm", name="mem")
                # single DoubleRow matmul: [Tprev;Tcur].T @ [u_{t-1}|u_t]
                nc.tensor.matmul(
                    mem[:, 0:M], tt_sb[:, 0:2, :],
                    u_all[:, (t_idx - 1) * M:(t_idx + 1) * M].rearrange(
                        "p (k m) -> p k m", k=2),
                    start=True, stop=True, perf_mode=DR)
                rtr = p_rt.tile([TS, M], bf, tag="rtr")
                if st["qs"][j] is not None:
                    nc.vector.tensor_mul(rtr[:], st["qs"][j][:], mem[:])
                else:
                    # q LN-apply fused in: rtr = (zq * rs_q) * mem
                    nc.vector.scalar_tensor_tensor(
                        rtr[:], st["stashes"][j][:, 0, :],
                        st["rs"][:, 2 * j:2 * j + 1], mem[:],
                        ALU.mult, ALU.mult)
                rtrs.append(rtr)  # 64x-scaled (SCL inside the T matrices)
            for j in range(gn):
                if rtrs[j] is None:
                    rTs.append(None)
                    continue
                ps = p_pt.tile([TS, 4 * TS], bf, tag="pt", name="ps_r")
                for mb in range(4):
                    nc.tensor.transpose(ps[:, mb * TS:(mb + 1) * TS],
                                        rtrs[j][:, mb * TS:(mb + 1) * TS],
                                        id_sb[:])
                rT = p_rt.tile([TS, 4 * TS], fp8, tag="rT")
                nc.scalar.copy(rT[:], ps[:])
                rTs.append(rT)
            for j in range(gn):
                if rTs[j] is None:
                    continue
                t_idx = g0 + j
                rTr = rTs[j].rearrange("p (k w) -> p k w", k=4)
                # g-outer over both 512-col halves: the rT stationary is
                # shared, so its expensive first LDWEIGHTS amortizes
                pss = [p_out.tile([TS, 512], f32, tag="out", name="out_ps")
                       for _ in range(2)]
                for g in range(2):
                    for nb in range(2):
                        nc.tensor.matmul(
                            pss[nb][:],
                            rTr[:, 2 * g:2 * g + 2, 0:TS],
                            wu_sb[:, 2 * g:2 * g + 2,
                                  nb * 512:(nb + 1) * 512],
                            start=(g == 0), stop=(g == 1), perf_mode=DR)
                for nb in range(2):
                    cols = slice(nb * 512, (nb + 1) * 512)
                    y_sb = p_y.tile([TS, 512], bf, tag="y")
                    # y = x + out/SCL  (out carries the 64x mem scaling)
                    nc.vector.scalar_tensor_tensor(
                        y_sb[:], pss[nb][:], 1.0 / SCL, xfs[j][:, cols],
                        ALU.mult, ALU.add)
                    if has_bu:
                        nc.vector.tensor_add(y_sb[:], y_sb[:],
                                             opt_sb["bu_b"][:, cols])
                    nc.sync.dma_start(y[(t_idx - 1) * TS:t_idx * TS, cols],
                                      y_sb[:])

        # software pipeline: K(g-1), A(g), B(g-1), A3(g), ...
        # A3 (newton + diag builds) MUST come after B(g-1): its newton
        # depends on A(g)'s last stats, and anything queued behind it on
        # DVE would stall -- B(g-1)'s u/rtr/y evacuations gate B's PE.
        prev = None
        xt_load(0)
        # rest of the weight pack, behind Wd + xT(0) in the queues
        nc.sync.dma_start(wp_sb[:, WD_COLS:], wpack[:, WD_COLS:])
        for gi, (g0, gn) in enumerate(GROUPS):
            if prev is not None:
                phase_a2(prev)
            xt_load(gi + 1)
            cur = phase_a(gi, g0, gn)
            if prev is not None:
                phase_b(prev)
            phase_a3(cur)
            prev = cur
        phase_a2(prev)
        phase_b(prev)

    _fix_matmult_waits(nc)
    return nc


def _prep_inputs(x, Wd, bd, Wq, bq, Wk, bk, Wv, bv, gq, bq_ln, gk, bk_ln,
                 W1, W2, Wu, bu, adaptive_lr, forget_factor):
    """Host-side: flags, decay matrices, per-core slabs, bf16 packing."""
    f = np.float32
    bd, bq, bk, bv, bu = (np.asarray(a, f) for a in (bd, bq, bk, bv, bu))
    gq, bq_ln, gk, bk_ln = (np.asarray(a, f) for a in (gq, bq_ln, gk, bk_ln))
    # mean-fold: LN subtracts the mean, so project Wq/Wk (and bq/bk) onto
    # zero-column-mean space host-side -- the on-device mean term vanishes
    Wq = np.asarray(Wq, f) - np.mean(np.asarray(Wq, f), axis=1, keepdims=True)
    Wk = np.asarray(Wk, f) - np.mean(np.asarray(Wk, f), axis=1, keepdims=True)
    bq = bq - bq.mean()
    bk = bk - bk.mean()
    flags = (bool(bd.any()), bool(bq.any()), bool(bk.any()), bool(bv.any()),
             bool((gq != 1).any()), bool(bq_ln.any()),
             bool((gk != 1).any()), bool(bk_ln.any()), bool(bu.any()))

    g = 1.0 / (1.0 + np.exp(-np.float64(forget_factor)))
    lr = np.float64(adaptive_lr)
    t_idx = np.arange(TS)
    lag_cur = t_idx[:, None] - t_idx[None, :]
    Tcur = np.where(lag_cur >= 0, g ** np.maximum(lag_cur, 0), 0.0) * lr * SCL
    lag_prev = t_idx[:, None] + TS - t_idx[None, :]
    Tprev = (g ** lag_prev) * lr * SCL
    TT = np.concatenate([Tprev, Tcur], axis=1).T.astype(f)  # [256, 128]

    def seg(w):
        w = np.asarray(w, f)          # [K, N] -> [128, nk*N]
        nk = w.shape[0] // TS
        return w.reshape(nk, TS, w.shape[1]).transpose(1, 0, 2).reshape(TS, -1)

    wpack = np.ascontiguousarray(np.concatenate(
        [seg(w) for w in (Wd, Wq, Wk, Wv, W1, -np.asarray(W2, f), Wu, TT)],
        axis=1)).astype(np_fp8)
    common = {
        "wpack": wpack,
        "ident": np.eye(TS, dtype=f).astype(np_bf16),
    }
    names = ("bd_c", "bq_b", "bk_b", "bv_b", "gq_b", "bqln_b", "gkT",
             "bklnT", "bu_b")
    vecs = (bd, bq, bk, bv, gq, bq_ln, gk, bk_ln, bu)
    for name, used, vec in zip(names, flags, vecs):
        if not used:
            continue
        if name in ("bd_c", "gkT", "bklnT"):
            common[name] = np.ascontiguousarray(
                vec.reshape(4, TS).T, f)      # [128, 4]: col mb = block
        else:
            common[name] = np.ascontiguousarray(
                np.broadcast_to(vec, (TS, vec.shape[0])), f)

    x = np.asarray(x, f)
    in_maps = []
    for c in range(N_CORES):
        b, sh = c // 2, c % 2
        if sh == 0:
            haloblk = np.zeros((TS, D), f)
            hm = np.zeros((TS, 1), f)
        else:
            haloblk = x[b, HALF - TS:HALF]
            hm = np.ones((TS, 1), f)
        slab = np.concatenate([haloblk, x[b, sh * HALF:(sh + 1) * HALF]],
                              axis=0)
        m = dict(common)
        # [SLAB, D] -> transpose -> [8, 128, SLAB] -> [128, 8*SLAB] fp8
        xt = np.ascontiguousarray(slab.T).reshape(8, TS, SLAB)
        m["x_t8"] = np.ascontiguousarray(
            xt.transpose(1, 0, 2).reshape(TS, 8 * SLAB)).astype(np_fp8)
        m["x_bf"] = np.ascontiguousarray(
            x[b, sh * HALF:(sh + 1) * HALF]).astype(np_bf16)
        m["hmask"] = hm
        in_maps.append(m)
    return flags, in_maps


def kernel(**inputs):
    global LAST_RESULTS
    flags, in_maps = _prep_inputs(**inputs)
    if flags not in _PROG_CACHE:
        _PROG_CACHE[flags] = _build_program(flags)
    nc = _PROG_CACHE[flags]

    res = run_bass_kernel_spmd(nc, in_maps, list(range(N_CORES)),
                               trace=TRACE, trace_kwargs=TRACE_KWARGS)
    LAST_RESULTS = res

    out = np.empty((B, S, D), np.float32)
    for c in range(N_CORES):
        b, sh = c // 2, c % 2
        out[b, sh * HALF:(sh + 1) * HALF] = res.results[c]["y"].astype(
            np.float32)
    return out
